# revision 1
# baseline (speedup 1.0000x reference)
"""LIF spike recurrence kernel for Trainium2 (8 NeuronCores, SPMD).

Problem: x [32, 128, 32, 32, 8] f32, recurrence over last (time) dim:
    u_t = TAU * u_{t-1} * (1 - o_{t-1}) + x_t
    o_t = 1[u_t - VTH > 0]
Output: o [32, 128, 32, 32, 8] f32 (0.0 / 1.0 spikes).

Strategy:
  - Shard batch dim (32) across 8 cores -> 4 per core. Pure elementwise over
    pixels; no communication.
  - Per core: 524288 pixels x 8 timesteps, viewed as DRAM [128, 4096, 8]
    (partition-major, each partition one contiguous 128KB run).
  - Tiles [128, F, 8]. Recurrence runs IN PLACE in the interleaved tile:
    membrane u_t overwrites x_t slice via two fused scalar_tensor_tensor ops
    per step; a single contiguous 2x-mode tensor_scalar pass converts the
    whole tile of membrane potentials into spikes at the end.
  - Exact fp32 equivalence with the reference:
      mask = (u <= VTH) in {0.0, 1.0};  c = mask*u;  u' = c*0.25 + x_t
    gives bitwise the same values as TAU*u*(1-o)+x_t (mult by 0/1 and by
    2^-2 are exact), and (u > VTH) == (u - VTH > 0) in fp32 because the
    rounded difference never flips sign (Sterbenz + magnitude arguments).
"""

import numpy as np

TAU = 0.25
VTH = 0.3
N_CORES = 8
P = 128
T = 8
B_LOC = 4  # batches per core
PIX_PER_CORE = B_LOC * 128 * 32 * 32  # 524288
NPP = PIX_PER_CORE // P  # 4096 pixels per partition
F = 1024  # pixels per partition per tile
N_TILES = NPP // F

_CACHE = {}

# Per-tile engine plan: (stt_engine, thr_engine).
#   stt: "vector" | "gpsimd"        — engine for the 14 recurrence STT ops
#   thr: "vector" | "gpsimd" | "scalar" — engine for the final spike threshold
#   (scalar = ACT does relu(sign(u - VTH)), exact in fp32)
PLAN = [("vector", "scalar")] * N_TILES
XP_BUFS = 3
POOL_PROBE = 0  # emit N idle GpSimd strided tensor_copy ops (cost probe)
LAYOUT = "planes"  # "inplace" (v1/v2) | "planes" (contiguous membrane scratch)
# Non-uniform tile sizes (pixels per partition, must sum to NPP): small first
# tile shortens the pipeline head, small last tile shortens the tail.
TILE_SIZES = [512, 1024, 1024, 1024, 512]


def _emit_tile_planes(nc, tc, mybir, xp, wp, cp, x_d, o_d, off, fi, dve_out=False):
    """Contiguous-membrane variant: u_t lives in plane-major scratch w so the
    recurrence STT ops read/write contiguously (only the x_t read is strided);
    ACT re-interleaves via per-plane Sign (strided write) + one big Relu."""
    Alu = mybir.AluOpType
    AF = mybir.ActivationFunctionType
    f32 = mybir.dt.float32

    xt = xp.tile([P, fi, T], f32, tag="xt")
    nc.sync.dma_start(xt[:], x_d[:, off : off + fi, :])
    w = wp.tile([P, T, fi], f32, tag="w")
    # u_0 = x_0
    nc.vector.tensor_copy(w[:, 0, :], xt[:, :, 0])
    for t in range(1, T):
        c = cp.tile([P, fi], f32, tag="c")
        # c = (u_{t-1} <= VTH) * u_{t-1}       (all contiguous)
        nc.vector.scalar_tensor_tensor(
            c[:], w[:, t - 1, :], VTH, w[:, t - 1, :], op0=Alu.is_le, op1=Alu.mult
        )
        # u_t = c * TAU + x_t                  (one strided read)
        nc.vector.scalar_tensor_tensor(
            w[:, t, :], c[:], TAU, xt[:, :, t], op0=Alu.mult, op1=Alu.add
        )
    if dve_out:
        # Tail tile: DVE is idle here; is_gt writes {0,1} directly (no relu).
        for t in range(T):
            nc.vector.tensor_scalar(
                xt[:, :, t], w[:, t, :], VTH, None, op0=Alu.is_gt
            )
    else:
        # Spike: sign(u_t - VTH) written interleaved, then one big in-place Relu.
        for t in range(T):
            nc.scalar.activation(xt[:, :, t], w[:, t, :], AF.Sign, bias=-VTH)
        flat = xt.rearrange("p f t -> p (f t)")
        nc.scalar.activation(flat, flat, AF.Relu)
    nc.sync.dma_start(o_d[:, off : off + fi, :], xt[:])


def _emit_tile(nc, tc, mybir, xp, cp, x_d, o_d, i, stt_engine, thr_engine):
    Alu = mybir.AluOpType
    f32 = mybir.dt.float32

    xt = xp.tile([P, F, T], f32, tag="xt")
    nc.sync.dma_start(xt[:], x_d[:, i * F : (i + 1) * F, :])
    # u_0 = x_0 is already in place at slice 0.
    if stt_engine == "gpsimd3op":
        # Pool has no scalar_tensor_tensor; equivalent 3-op form.
        # m = (u <= VTH) * TAU  in {0, TAU};  c = u * m;  u' = c + x_t
        g = nc.gpsimd
        for t in range(1, T):
            up = xt[:, :, t - 1]
            m = cp.tile([P, F], f32, tag="m")
            g.tensor_scalar(m[:], up, VTH, TAU, op0=Alu.is_le, op1=Alu.mult)
            c = cp.tile([P, F], f32, tag="c")
            g.tensor_tensor(c[:], up, m[:], op=Alu.mult)
            g.tensor_tensor(xt[:, :, t], c[:], xt[:, :, t], op=Alu.add)
    else:
        eng = {"vector": nc.vector, "gpsimd": nc.gpsimd}[stt_engine]
        for t in range(1, T):
            up = xt[:, :, t - 1]
            c = cp.tile([P, F], f32, tag="c")
            # c = (u_prev <= VTH) * u_prev
            eng.scalar_tensor_tensor(c[:], up, VTH, up, op0=Alu.is_le, op1=Alu.mult)
            # u_t = c * TAU + x_t   (in place over x_t slice)
            eng.scalar_tensor_tensor(
                xt[:, :, t], c[:], TAU, xt[:, :, t], op0=Alu.mult, op1=Alu.add
            )
    # Whole-tile spike threshold, contiguous, in place.
    flat = xt.rearrange("p f t -> p (f t)")
    if thr_engine == "scalar":
        AF = mybir.ActivationFunctionType
        nc.scalar.activation(flat, flat, AF.Sign, bias=-VTH, scale=1.0)
        nc.scalar.activation(flat, flat, AF.Relu)
    else:
        teng = {"vector": nc.vector, "gpsimd": nc.gpsimd}[thr_engine]
        teng.tensor_scalar(flat, flat, VTH, None, op0=Alu.is_gt)
    nc.sync.dma_start(o_d[:, i * F : (i + 1) * F, :], xt[:])


def _build_nc():
    import concourse.tile as tile
    from concourse import bacc, mybir

    f32 = mybir.dt.float32

    nc = bacc.Bacc(
        "TRN2",
        target_bir_lowering=False,
        debug=False,
        enable_asserts=False,
        num_devices=N_CORES,
    )
    x_d = nc.dram_tensor("x", [P, NPP, T], f32, kind="ExternalInput").ap()
    o_d = nc.dram_tensor("o", [P, NPP, T], f32, kind="ExternalOutput").ap()

    if LAYOUT == "planes" or any(thr == "scalar" for _, thr in PLAN):
        # ACT activation bias needs a pre-registered const AP.
        cb = nc.alloc_sbuf_tensor("const-f32-negvth", [128, 1], f32)
        nc.gpsimd.memset(cb.ap(), -VTH)
        nc.const_aps.aps[(f32, -VTH)] = cb.ap()
        nc.all_engine_barrier()

    with tile.TileContext(nc) as tc:
        with tc.tile_pool(name="xp", bufs=XP_BUFS) as xp, tc.tile_pool(
            name="cp", bufs=2
        ) as cp, tc.tile_pool(name="wp", bufs=2) as wp:
            if LAYOUT == "planes":
                assert sum(TILE_SIZES) == NPP
                off = 0
                for k, fi in enumerate(TILE_SIZES):
                    _emit_tile_planes(
                        nc, tc, mybir, xp, wp, cp, x_d, o_d, off, fi,
                        dve_out=(k == len(TILE_SIZES) - 1),
                    )
                    off += fi
            else:
                for i in range(N_TILES):
                    stt_e, thr_e = PLAN[i]
                    _emit_tile(nc, tc, mybir, xp, cp, x_d, o_d, i, stt_e, thr_e)
            if POOL_PROBE:
                # Idle-engine cost probe: strided + contig copies on GpSimd
                # over scratch (results unused, races impossible).
                with tc.tile_pool(name="probe", bufs=1) as sp:
                    ptile = sp.tile([P, F, T], f32)
                    nc.gpsimd.memset(ptile.rearrange("p f t -> p (f t)"), 0.0)
                    for r in range(POOL_PROBE):
                        nc.gpsimd.tensor_copy(
                            ptile[:, :, (r % (T - 1)) + 1], ptile[:, :, 0]
                        )
    nc.compile()
    return nc


def _get_nc():
    if "nc" not in _CACHE:
        _CACHE["nc"] = _build_nc()
    return _CACHE["nc"]


def _shard(x: np.ndarray):
    xs = np.ascontiguousarray(x, dtype=np.float32)
    return [
        np.ascontiguousarray(xs[i * B_LOC : (i + 1) * B_LOC].reshape(P, NPP, T))
        for i in range(N_CORES)
    ]


def _run(in_maps, **kwargs):
    from concourse.bass_utils import run_bass_kernel_spmd

    nc = _get_nc()
    return run_bass_kernel_spmd(nc, in_maps, core_ids=list(range(N_CORES)), **kwargs)


def kernel(x: np.ndarray) -> np.ndarray:
    in_maps = [{"x": s} for s in _shard(x)]
    res = _run(in_maps)
    outs = [
        res.results[i]["o"].reshape(B_LOC, 128, 32, 32, T) for i in range(N_CORES)
    ]
    return np.concatenate(outs, axis=0)

